# revision 12
# baseline (speedup 1.0000x reference)
"""CTNet forward on 8 Trainium2 NeuronCores, data-parallel over batch.

B=16 graphs of N=1024 nodes; 2 graphs per core. The heavy per-graph work
(Laplacian/ct-rewiring traces, cdist-based adjacency rewiring, graph convs,
mincut pooling contractions over N) runs on device; the tiny [16,16]-scale
tail (cluster-graph conv2, MLP head, scalar losses) finishes on host.

Device dataflow per graph (feature-major activations [feat, node]):
  X0^T via PE transposes -> X1^T = W1^T X0^T (+b1) -> S1^T -> tanh -> S^T
  A tiles -> 64 PE transposes -> A^T (f32r) ; Asq = A^T**2
  ASt = [S_nm|1]^T A^T = [(A S)^T ; rowsum(A)]  -> trace terms, vol, basedot
  d2 = one fused K=66 matmul per tile ([-2S^T;rn2;1] x [S^T;1;rn2])
  Anew^T = sqrt(max(d2,0) * Asq) / vol   (clamp+mul on DVE, sqrt on ACT)
  R^T = [X1|1]_nm^T Anew^T = [(Anew X1)^T ; rowsum(Anew)]
  X2^T = Wrel1^T R^T + brel1 + Wroot1^T X1^T ;  S2 = softmax_nm(X2 Wp2 + bp2)
  P^T = S2_nm^T Anew^T = (Anew S2)^T ;  SS1 = S^T S
Matmuls with N>=256 use float32r (full PE rate, ~1e-4 rel err); transposes
are exact f32. f32r matmul operands are produced by DVE/ACT ops (the walrus
verifier requires rounded producers); rows that need cross-partition
placement are staged in f32 via DMA and rounded with one whole-tile copy.
"""

import numpy as np

import concourse.bass as bass
import concourse.mybir as mybir
from concourse.tile import TileContext
from concourse.bass_utils import run_bass_kernel_spmd

dt = mybir.dt
AF = mybir.ActivationFunctionType
ALU = mybir.AluOpType

EPS = 1e-15
B, N, FIN, H, K1, K2, OUT = 16, 1024, 128, 32, 64, 16, 10
NC = 8
BL = B // NC  # graphs per core
NT = N // 128  # node tiles

_CACHE = {}


def _split_excess_waits(nc):
    """Hoist excess sync waits into standalone EventSemaphore instructions.

    This walrus build rejects >1 wait on Matmult (self-loading f32/f32r
    weights, S3_LW struct) and on Drain (CTRL_NO struct). A wait hoisted to
    an earlier same-engine instruction is semantically identical.
    """
    import bass_rust

    limit = 1  # this walrus build allows a single wait on every struct
    for f in nc.m.functions:
        for b in f.blocks:
            new_list = []
            for inst in b.instructions:
                si = getattr(inst, "sync_info", None)
                if si is not None:
                    waits = list(si.on_wait)
                    if len(waits) > limit:
                        extras, keep = waits[:-limit], waits[-limit:]
                        for k, w in enumerate(extras):
                            ev = bass_rust.InstEventSemaphore(name=f"{inst.name}-w{k}")
                            ev.engine = inst.engine
                            ev.sync_info = bass_rust.SyncInfo(on_wait=[w], on_update=[])
                            new_list.append(ev)
                        inst.sync_info = bass_rust.SyncInfo(
                            on_wait=keep, on_update=list(si.on_update)
                        )
                new_list.append(inst)
            b.instructions = new_list


def _build():
    nc = bass.Bass()
    f32, f32r = dt.float32, dt.float32r

    a_loc = nc.declare_dram_parameter("a_loc", [BL, N, N], f32, isOutput=False)
    x_loc = nc.declare_dram_parameter("x_loc", [BL, N, FIN], f32, isOutput=False)
    w1_d = nc.declare_dram_parameter("w1", [FIN, H], f32, isOutput=False)
    b1_d = nc.declare_dram_parameter("b1", [H, 1], f32, isOutput=False)
    wp1_d = nc.declare_dram_parameter("wp1", [H, K1], f32, isOutput=False)
    bp1_d = nc.declare_dram_parameter("bp1", [K1, 1], f32, isOutput=False)
    wrel1_d = nc.declare_dram_parameter("wrel1", [H, H], f32, isOutput=False)
    brel1_d = nc.declare_dram_parameter("brel1", [1, H], f32, isOutput=False)
    wroot1_d = nc.declare_dram_parameter("wroot1", [H, H], f32, isOutput=False)
    wp2a_d = nc.declare_dram_parameter("wp2aug", [H + 1, K2], f32, isOutput=False)
    ident_d = nc.declare_dram_parameter("ident", [128, 128], f32, isOutput=False)

    o_x2t = nc.declare_dram_parameter("o_x2t", [BL, H, N], f32, isOutput=True)
    o_s2 = nc.declare_dram_parameter("o_s2", [BL, N, K2], f32, isOutput=True)
    o_pt = nc.declare_dram_parameter("o_pt", [BL, K2, N], f32, isOutput=True)
    o_df2 = nc.declare_dram_parameter("o_df2", [BL, 1, N], f32, isOutput=True)
    o_ss1 = nc.declare_dram_parameter("o_ss1", [BL, K1, K1], f32, isOutput=True)
    o_aux = nc.declare_dram_parameter("o_aux", [BL, K1, 8], f32, isOutput=True)
    o_vb = nc.declare_dram_parameter("o_vb", [BL, 1, 8], f32, isOutput=True)

    with TileContext(nc) as tc:
        with (
            tc.tile_pool(name="consts", bufs=1) as consts,
            tc.tile_pool(name="abig", bufs=3) as abig,
            tc.tile_pool(name="atr", bufs=1) as p_atr,
            tc.tile_pool(name="anew", bufs=1) as p_anew,
            tc.tile_pool(name="acts", bufs=1) as acts,
            tc.tile_pool(name="stage", bufs=1) as stage,
            tc.tile_pool(name="scr", bufs=2) as scr,
            tc.tile_pool(name="ps_tr", bufs=3, space="PSUM") as ps_tr,
            tc.tile_pool(name="ps_d2", bufs=2, space="PSUM") as ps_d2,
            tc.tile_pool(name="ps_acc", bufs=2, space="PSUM") as ps_acc,
            tc.tile_pool(name="ps_mm", bufs=1, space="PSUM") as ps_mm,
        ):
            ident = consts.tile([128, 128], f32, tag="ident")
            nc.sync.dma_start(out=ident[:], in_=ident_d[:])

            def rounded_const(name, shape, src_dram):
                tf = consts.tile(shape, f32, tag=name + "f")
                nc.sync.dma_start(out=tf[:], in_=src_dram[:])
                tr = consts.tile(shape, f32r, tag=name)
                nc.vector.tensor_copy(tr[:], tf[:])
                return tr

            w1 = rounded_const("w1", [FIN, H], w1_d)
            wp1 = rounded_const("wp1", [H, K1], wp1_d)
            wrel1 = rounded_const("wrel1", [H, H], wrel1_d)
            wroot1 = rounded_const("wroot1", [H, H], wroot1_d)
            brel1 = rounded_const("brel1", [1, H], brel1_d)
            wp2a = rounded_const("wp2a", [H + 1, K2], wp2a_d)
            b1c = consts.tile([H, 1], f32, tag="b1c")
            nc.sync.dma_start(out=b1c[:], in_=b1_d[:])
            bp1c = consts.tile([K1, 1], f32, tag="bp1c")
            nc.sync.dma_start(out=bp1c[:], in_=bp1_d[:])

            ones_f = consts.tile([1, N], f32, tag="ones_f")
            nc.vector.memset(ones_f[:], 1.0)
            ones_row = consts.tile([1, N], f32r, tag="ones_row")
            nc.vector.tensor_copy(ones_row[:], ones_f[:])
            ones64f = consts.tile([K1, 1], f32, tag="ones64f")
            nc.vector.memset(ones64f[:], 1.0)
            ones64 = consts.tile([K1, 1], f32r, tag="ones64")
            nc.vector.tensor_copy(ones64[:], ones64f[:])
            onescol = consts.tile([128, 1], f32, tag="onescol")
            nc.vector.memset(onescol[:], 1.0)

            for g in range(BL):
                # ---- X0^T (feature-major input) ----
                x0t = acts.tile([FIN, N], f32r, tag="x0t")
                for j in range(NT):
                    x0 = abig.tile([128, FIN], f32, tag="x0")
                    nc.sync.dma_start(out=x0[:], in_=x_loc[g, j * 128 : (j + 1) * 128, :])
                    pt_ = ps_tr.tile([128, 128], f32, tag="ptr")
                    nc.tensor.transpose(pt_[:], x0[:], ident[:])
                    nc.vector.tensor_copy(x0t[:, j * 128 : (j + 1) * 128], pt_[:])

                # ---- X1^T = W1^T X0^T + b1 (staged f32, then rounded) ----
                x1t_f = stage.tile([H + 1, N], f32, tag="x1t_f")
                nc.vector.memset(x1t_f[H : H + 1, :], 1.0)
                for c in range(2):
                    pm = ps_mm.tile([H, 512], f32, tag="pmm")
                    nc.tensor.matmul(
                        pm[:], w1[:], x0t[:, c * 512 : (c + 1) * 512],
                        start=True, stop=True,
                    )
                    nc.vector.tensor_scalar_add(
                        x1t_f[0:H, c * 512 : (c + 1) * 512], pm[:], b1c[:]
                    )
                x1taug = acts.tile([H + 1, N], f32r, tag="x1taug")
                nc.vector.tensor_copy(x1taug[:], x1t_f[:])

                # ---- S^T = tanh(Wp1^T X1^T + bp1); colsum via accum ----
                st_f = stage.tile([K1 + 2, N], f32, tag="st_f")
                sts_f = stage.tile([K1 + 2, N], f32, tag="sts_f")
                aux = scr.tile([K1, 8], f32, tag="aux")
                for c in range(2):
                    pm = ps_mm.tile([K1, 512], f32, tag="pmm")
                    nc.tensor.matmul(
                        pm[:], wp1[:], x1taug[0:H, c * 512 : (c + 1) * 512],
                        start=True, stop=True,
                    )
                    nc.scalar.activation(
                        st_f[0:K1, c * 512 : (c + 1) * 512], pm[:], AF.Tanh,
                        bias=bp1c[:], accum_out=aux[:, c : c + 1],
                    )
                nc.vector.tensor_scalar_mul(sts_f[0:K1, :], st_f[0:K1, :], -2.0)

                # rn2 = sum_k S^2 as a [1,N] row (partition 0)
                stsq = stage.tile([K1, N], f32r, tag="stsq")
                nc.vector.tensor_mul(stsq[:], st_f[0:K1, :], st_f[0:K1, :])
                rn2_sb = stage.tile([1, N], f32, tag="rn2_sb")
                for c in range(2):
                    pm = ps_mm.tile([1, 512], f32, tag="pmm")
                    nc.tensor.matmul(
                        pm[:], ones64[:], stsq[:, c * 512 : (c + 1) * 512],
                        start=True, stop=True,
                    )
                    nc.scalar.activation(
                        rn2_sb[:, c * 512 : (c + 1) * 512], pm[:], AF.Copy
                    )
                # special rows: st = [S; 1; rn2], sts = [-2S; rn2; 1]
                nc.vector.memset(st_f[K1 : K1 + 1, :], 1.0)
                nc.sync.dma_start(out=st_f[K1 + 1 : K1 + 2, :], in_=rn2_sb[:])
                nc.sync.dma_start(out=sts_f[K1 : K1 + 1, :], in_=rn2_sb[:])
                nc.sync.dma_start(out=sts_f[K1 + 1 : K1 + 2, :], in_=ones_f[:])
                st = acts.tile([K1 + 2, N], f32r, tag="st")
                sts = acts.tile([K1 + 2, N], f32r, tag="sts")
                nc.vector.tensor_copy(st[:], st_f[:])
                nc.vector.tensor_copy(sts[:], sts_f[:])

                # ---- S node-major blocks + ones col (Zstat) ----
                zstat = []
                for j in range(NT):
                    zs = acts.tile([128, K1 + 1], f32r, tag=f"zstat{j}")
                    pt_ = ps_tr.tile([128, 128], f32, tag="ptr")
                    nc.tensor.transpose(
                        pt_[0:128, 0:K1],
                        st_f[0:K1, j * 128 : (j + 1) * 128],
                        ident[0:K1, 0:K1],
                    )
                    nc.vector.tensor_copy(zs[:, 0:K1], pt_[0:128, 0:K1])
                    nc.vector.tensor_copy(zs[:, K1 : K1 + 1], onescol[:])
                    zstat.append(zs)

                # ---- A^T tiles (PE transpose) + Asq ----
                atr = []
                for j in range(NT):
                    at_j = p_atr.tile([128, N], f32r, tag=f"atr{j}")
                    atr.append(at_j)
                for i in range(NT):
                    a_nat = abig.tile([128, N], f32, tag="a_nat")
                    nc.sync.dma_start(
                        out=a_nat[:], in_=a_loc[g, i * 128 : (i + 1) * 128, :]
                    )
                    for j in range(NT):
                        pt_ = ps_tr.tile([128, 128], f32, tag="ptr")
                        nc.tensor.transpose(
                            pt_[:], a_nat[:, j * 128 : (j + 1) * 128], ident[:]
                        )
                        nc.scalar.activation(
                            atr[j][:, i * 128 : (i + 1) * 128], pt_[:], AF.Copy
                        )

                # ---- ASt = [S_nm|1]^T A^T : rows 0..63 = (A S)^T, row 64 = dflat
                dfl = stage.tile([K1 + 1, N], f32, tag="dfl")
                vb = scr.tile([K1 + 1, 8], f32, tag="vb")
                for c in range(2):
                    pa = ps_acc.tile([K1 + 1, 512], f32, tag="pacc")
                    for j in range(NT):
                        nc.tensor.matmul(
                            pa[:], zstat[j][:], atr[j][:, c * 512 : (c + 1) * 512],
                            start=(j == 0), stop=(j == NT - 1),
                        )
                    ttr_dump = scr.tile([K1, 512], f32, tag="ttr_dump")
                    nc.vector.tensor_mul(
                        ttr_dump[:], pa[0:K1, :],
                        st[0:K1, c * 512 : (c + 1) * 512].bitcast(f32),
                    )
                    nc.vector.reduce_sum(
                        aux[:, 2 + c : 3 + c], ttr_dump[:], axis=mybir.AxisListType.X
                    )
                    nc.scalar.activation(
                        dfl[K1 : K1 + 1, c * 512 : (c + 1) * 512],
                        pa[K1 : K1 + 1, :],
                        AF.Copy, accum_out=vb[K1 : K1 + 1, c : c + 1],
                    )
                # basedot = sum(dflat * rn2) ; rn2 lives at sts row 64
                bd_dump = stage.tile([K1 + 1, N], f32, tag="bd_dump")
                nc.vector.tensor_mul(
                    bd_dump[K1 : K1 + 1, :], dfl[K1 : K1 + 1, :],
                    sts[K1 : K1 + 1, :].bitcast(f32),
                )
                nc.vector.reduce_sum(
                    vb[K1 : K1 + 1, 2:3], bd_dump[K1 : K1 + 1, :],
                    axis=mybir.AxisListType.X,
                )
                # vol = volp0 + volp1 + N*EPS -> invvol2 broadcast [128,1]
                nc.vector.tensor_tensor(
                    out=vb[K1 : K1 + 1, 3:4], in0=vb[K1 : K1 + 1, 0:1],
                    in1=vb[K1 : K1 + 1, 1:2], op=ALU.add,
                )
                nc.vector.tensor_scalar_add(
                    vb[K1 : K1 + 1, 4:5], vb[K1 : K1 + 1, 3:4], float(N) * EPS
                )
                nc.sync.dma_start(out=o_vb[g], in_=vb[K1 : K1 + 1, :])
                volp0 = scr.tile([1, 2], f32, tag="volp0")
                nc.sync.dma_start(out=volp0[:, 0:1], in_=vb[K1 : K1 + 1, 4:5])
                pv = ps_mm.tile([128, 1], f32, tag="pmm")
                nc.tensor.matmul(
                    pv[:], ones_f[0:1, 0:128], volp0[:, 0:1], start=True, stop=True
                )
                rcp = scr.tile([128, 2], f32, tag="rcp")
                nc.vector.reciprocal(rcp[:, 0:1], pv[:])
                nc.vector.tensor_mul(rcp[:, 1:2], rcp[:, 0:1], rcp[:, 0:1])

                # ---- d2 -> clamp*Asq -> sqrt -> Anew^T (f32r) ----
                anew = []
                for j in range(NT):
                    an_j = p_anew.tile([128, N], f32r, tag=f"anew{j}")
                    anew.append(an_j)
                for j in range(NT):
                    for c in range(2):
                        pd = ps_d2.tile([128, 512], f32, tag="pd2")
                        nc.tensor.matmul(
                            pd[:],
                            sts[:, j * 128 : (j + 1) * 128],
                            st[:, c * 512 : (c + 1) * 512],
                            start=True, stop=True,
                        )
                        tq = scr.tile([128, 512], f32, tag="tq")
                        nc.vector.scalar_tensor_tensor(
                            out=tq[:], in0=pd[:], scalar=0.0,
                            in1=atr[j][:, c * 512 : (c + 1) * 512].bitcast(f32),
                            op0=ALU.max, op1=ALU.mult,
                        )
                        tq2 = scr.tile([128, 512], f32, tag="tq2")
                        nc.vector.tensor_mul(
                            tq2[:], tq[:],
                            atr[j][:, c * 512 : (c + 1) * 512].bitcast(f32),
                        )
                        nc.scalar.activation(
                            anew[j][:, c * 512 : (c + 1) * 512], tq2[:], AF.Sqrt,
                            scale=rcp[:, 1:2],
                        )

                # ---- X1 node-major aug blocks ----
                x1nm = []
                for j in range(NT):
                    xn = acts.tile([128, H + 1], f32r, tag=f"x1nm{j}")
                    pt_ = ps_tr.tile([128, 128], f32, tag="ptr")
                    nc.tensor.transpose(
                        pt_[0:128, 0 : H + 1],
                        x1t_f[:, j * 128 : (j + 1) * 128],
                        ident[0 : H + 1, 0 : H + 1],
                    )
                    nc.vector.tensor_copy(xn[:], pt_[0:128, 0 : H + 1])
                    x1nm.append(xn)

                # ---- R^T = [X1|1]_nm^T Anew^T ----
                rt = acts.tile([H + 1, N], f32r, tag="rt")
                for c in range(2):
                    pa = ps_acc.tile([H + 1, 512], f32, tag="pacc")
                    for j in range(NT):
                        nc.tensor.matmul(
                            pa[:], x1nm[j][:], anew[j][:, c * 512 : (c + 1) * 512],
                            start=(j == 0), stop=(j == NT - 1),
                        )
                    nc.vector.tensor_copy(rt[:, c * 512 : (c + 1) * 512], pa[:])
                nc.sync.dma_start(out=o_df2[g], in_=rt[H : H + 1, :].bitcast(f32))

                # ---- X2^T = Wrel1^T R^T + brel1 + Wroot1^T X1^T ----
                x2t_f = stage.tile([H + 1, N], f32, tag="x2t_f")
                nc.vector.memset(x2t_f[H : H + 1, :], 1.0)
                for c in range(2):
                    pm = ps_mm.tile([H, 512], f32, tag="pmm")
                    sl = slice(c * 512, (c + 1) * 512)
                    nc.tensor.matmul(pm[:], wrel1[:], rt[0:H, sl], start=True, stop=False)
                    nc.tensor.matmul(
                        pm[:], brel1[:], ones_row[:, sl], start=False, stop=False
                    )
                    nc.tensor.matmul(
                        pm[:], wroot1[:], x1taug[0:H, sl], start=False, stop=True
                    )
                    nc.vector.tensor_copy(x2t_f[0:H, sl], pm[:])
                x2taug = acts.tile([H + 1, N], f32r, tag="x2taug")
                nc.vector.tensor_copy(x2taug[:], x2t_f[:])
                nc.sync.dma_start(out=o_x2t[g], in_=x2t_f[0:H, :])

                # ---- S2 = softmax_nodes(X2 Wp2 + bp2), node-major ----
                s2all = acts.tile([128, NT * K2], f32r, tag="s2all")
                for j in range(NT):
                    pm = ps_mm.tile([128, K2], f32, tag="pmm")
                    nc.tensor.matmul(
                        pm[:], x2taug[:, j * 128 : (j + 1) * 128], wp2a[:],
                        start=True, stop=True,
                    )
                    en = scr.tile([128, K2], f32, tag="en")
                    se = scr.tile([128, 2], f32, tag="se")
                    nc.scalar.activation(en[:], pm[:], AF.Exp, accum_out=se[:, 0:1])
                    nc.vector.reciprocal(se[:, 1:2], se[:, 0:1])
                    nc.vector.tensor_scalar_mul(
                        s2all[:, j * K2 : (j + 1) * K2], en[:], se[:, 1:2]
                    )
                    nc.sync.dma_start(
                        out=o_s2[g, j * 128 : (j + 1) * 128, :],
                        in_=s2all[:, j * K2 : (j + 1) * K2].bitcast(f32),
                    )

                # ---- P^T = S2_nm^T Anew^T ----
                ptile = acts.tile([K2, N], f32, tag="ptile")
                for c in range(2):
                    pa = ps_acc.tile([K2, 512], f32, tag="pacc")
                    for j in range(NT):
                        nc.tensor.matmul(
                            pa[:], s2all[:, j * K2 : (j + 1) * K2],
                            anew[j][:, c * 512 : (c + 1) * 512],
                            start=(j == 0), stop=(j == NT - 1),
                        )
                    nc.vector.tensor_copy(ptile[:, c * 512 : (c + 1) * 512], pa[:])
                nc.sync.dma_start(out=o_pt[g], in_=ptile[:])

                # ---- SS1 = S^T S ----
                pss = ps_mm.tile([K1, K1], f32, tag="pmm")
                for j in range(NT):
                    nc.tensor.matmul(
                        pss[:], zstat[j][:, 0:K1], zstat[j][:, 0:K1],
                        start=(j == 0), stop=(j == NT - 1),
                    )
                ss1 = scr.tile([K1, K1], f32, tag="ss1")
                nc.vector.tensor_copy(ss1[:], pss[:])
                nc.sync.dma_start(out=o_ss1[g], in_=ss1[:])
                nc.sync.dma_start(out=o_aux[g], in_=aux[:])

    _split_excess_waits(nc)
    return nc


def kernel(**inputs):
    x = np.asarray(inputs["x"], dtype=np.float32)
    adj = np.asarray(inputs["adj"], dtype=np.float32)
    mask = np.asarray(inputs["mask"])
    m = mask.astype(np.float32)  # all-ones in this problem; applied to x anyway

    if "nc" not in _CACHE:
        _CACHE["nc"] = _build()
    nc = _CACHE["nc"]

    wk = {
        "w1": np.ascontiguousarray(inputs["W_lin1"], dtype=np.float32),
        "b1": np.ascontiguousarray(np.asarray(inputs["b_lin1"], np.float32)[:, None]),
        "wp1": np.ascontiguousarray(inputs["W_pool1"], dtype=np.float32),
        "bp1": np.ascontiguousarray(np.asarray(inputs["b_pool1"], np.float32)[:, None]),
        "wrel1": np.ascontiguousarray(inputs["Wrel1"], dtype=np.float32),
        "brel1": np.ascontiguousarray(np.asarray(inputs["brel1"], np.float32)[None, :]),
        "wroot1": np.ascontiguousarray(inputs["Wroot1"], dtype=np.float32),
        "wp2aug": np.ascontiguousarray(
            np.vstack(
                [
                    np.asarray(inputs["W_pool2"], np.float32),
                    np.asarray(inputs["b_pool2"], np.float32)[None, :],
                ]
            )
        ),
        "ident": np.eye(128, dtype=np.float32),
    }
    xm = x * m[:, :, None]
    in_maps = []
    for c in range(NC):
        sl = slice(c * BL, (c + 1) * BL)
        in_maps.append(
            dict(
                a_loc=np.ascontiguousarray(adj[sl]),
                x_loc=np.ascontiguousarray(xm[sl]),
                **wk,
            )
        )

    results = run_bass_kernel_spmd(nc, in_maps, list(range(NC))).results

    # ---------------- host tail ----------------
    f64 = np.float64
    Wrel2 = np.asarray(inputs["Wrel2"], f64)
    brel2 = np.asarray(inputs["brel2"], f64)
    Wroot2 = np.asarray(inputs["Wroot2"], f64)
    W_lin2 = np.asarray(inputs["W_lin2"], f64)
    b_lin2 = np.asarray(inputs["b_lin2"], f64)
    W_lin3 = np.asarray(inputs["W_lin3"], f64)
    b_lin3 = np.asarray(inputs["b_lin3"], f64)

    logits = np.zeros((B, OUT), np.float32)
    ct_ratios, o1sq, mincuts, o2s = [], [], [], []
    i64 = np.eye(K1, dtype=f64)
    i16 = np.eye(K2, dtype=f64)
    for c in range(NC):
        r = results[c]
        for g in range(BL):
            bidx = c * BL + g
            auxv = r["o_aux"][g].astype(f64)
            vbv = r["o_vb"][g][0].astype(f64)
            colsum = auxv[:, 0] + auxv[:, 1]
            trAS = auxv[:, 2].sum() + auxv[:, 3].sum()
            basedot = vbv[2]
            epsterm = EPS * np.sum(colsum**2)
            ct_num = basedot + epsterm - trAS
            ct_den = basedot + epsterm + EPS
            ct_ratios.append(ct_num / ct_den)
            ss1 = r["o_ss1"][g].astype(f64)
            diff1 = ss1 / np.sqrt(np.sum(ss1 * ss1)) - i64
            o1sq.append(np.sum(diff1 * diff1))

            X2 = r["o_x2t"][g].astype(f64).T  # [N, H]
            S2 = r["o_s2"][g].astype(f64)  # [N, K2]
            P = r["o_pt"][g].astype(f64).T  # [N, K2]
            dflat2 = r["o_df2"][g][0].astype(f64) + EPS
            OUT2 = S2.T @ X2
            OA = S2.T @ P
            mincut_num = np.trace(OA)
            mincut_den = np.sum(dflat2 * np.sum(S2 * S2, axis=1))
            mincuts.append(-(mincut_num / mincut_den))
            SS2 = S2.T @ S2
            diff2 = SS2 / np.sqrt(np.sum(SS2 * SS2)) - i16 / np.sqrt(float(K2))
            o2s.append(np.sqrt(np.sum(diff2 * diff2)))

            OAp = OA * (1.0 - i16)
            dk = np.sqrt(OAp.sum(axis=-1) + EPS) + EPS
            OAn = OAp / dk[None, :] / dk[:, None]
            X3 = (OAn @ OUT2) @ Wrel2 + brel2 + OUT2 @ Wroot2
            gsum = X3.sum(axis=0)
            h = np.maximum(gsum @ W_lin2 + b_lin2, 0.0)
            lg = h @ W_lin3 + b_lin3
            lg = lg - (np.log(np.sum(np.exp(lg - lg.max()))) + lg.max())
            logits[bidx] = lg.astype(np.float32)

    loss1 = np.float32(np.mean(ct_ratios) + np.sqrt(np.sum(o1sq)))
    loss2 = np.float32(np.mean(mincuts) + np.mean(o2s))
    return logits, loss1, loss2


# revision 13
# speedup vs baseline: 1.0362x; 1.0362x over previous
"""CTNet forward on 8 Trainium2 NeuronCores, data-parallel over batch.

B=16 graphs of N=1024 nodes; 2 graphs per core. The heavy per-graph work
(Laplacian/ct-rewiring traces, cdist-based adjacency rewiring, graph convs,
mincut pooling contractions over N) runs on device; the tiny [16,16]-scale
tail (cluster-graph conv2, MLP head, scalar losses) finishes on host.

Device dataflow per graph (feature-major activations [feat, node]):
  X0^T via PE transposes -> X1^T = W1^T X0^T (+b1) -> S1^T -> tanh -> S^T
  A tiles -> 64 PE transposes -> A^T (f32r) ; Asq = A^T**2
  ASt = [S_nm|1]^T A^T = [(A S)^T ; rowsum(A)]  -> trace terms, vol, basedot
  d2 = one fused K=66 matmul per tile ([-2S^T;rn2;1] x [S^T;1;rn2])
  Anew^T = sqrt(max(d2,0) * Asq) / vol   (clamp+mul on DVE, sqrt on ACT)
  R^T = [X1|1]_nm^T Anew^T = [(Anew X1)^T ; rowsum(Anew)]
  X2^T = Wrel1^T R^T + brel1 + Wroot1^T X1^T ;  S2 = softmax_nm(X2 Wp2 + bp2)
  P^T = S2_nm^T Anew^T = (Anew S2)^T ;  SS1 = S^T S
Matmuls with N>=256 use float32r (full PE rate, ~1e-4 rel err); transposes
are exact f32. f32r matmul operands are produced by DVE/ACT ops (the walrus
verifier requires rounded producers); rows that need cross-partition
placement are staged in f32 via DMA and rounded with one whole-tile copy.
"""

import numpy as np

import concourse.bass as bass
import concourse.mybir as mybir
from concourse.tile import TileContext
from concourse.bass_utils import run_bass_kernel_spmd

dt = mybir.dt
AF = mybir.ActivationFunctionType
ALU = mybir.AluOpType

EPS = 1e-15
B, N, FIN, H, K1, K2, OUT = 16, 1024, 128, 32, 64, 16, 10
NC = 8
BL = B // NC  # graphs per core
NT = N // 128  # node tiles

_CACHE = {}


def _split_excess_waits(nc):
    """Hoist excess sync waits into standalone EventSemaphore instructions.

    This walrus build rejects >1 wait on Matmult (self-loading f32/f32r
    weights, S3_LW struct) and on Drain (CTRL_NO struct). A wait hoisted to
    an earlier same-engine instruction is semantically identical.
    """
    import bass_rust

    limit = 1  # this walrus build allows a single wait on every struct
    for f in nc.m.functions:
        for b in f.blocks:
            new_list = []
            for inst in b.instructions:
                si = getattr(inst, "sync_info", None)
                if si is not None:
                    waits = list(si.on_wait)
                    if len(waits) > limit:
                        extras, keep = waits[:-limit], waits[-limit:]
                        for k, w in enumerate(extras):
                            ev = bass_rust.InstEventSemaphore(name=f"{inst.name}-w{k}")
                            ev.engine = inst.engine
                            ev.sync_info = bass_rust.SyncInfo(on_wait=[w], on_update=[])
                            new_list.append(ev)
                        inst.sync_info = bass_rust.SyncInfo(
                            on_wait=keep, on_update=list(si.on_update)
                        )
                new_list.append(inst)
            b.instructions = new_list


def _build():
    nc = bass.Bass()
    f32, f32r = dt.float32, dt.float32r

    a_loc = nc.declare_dram_parameter("a_loc", [BL, N, N], f32, isOutput=False)
    x_loc = nc.declare_dram_parameter("x_loc", [BL, N, FIN], f32, isOutput=False)
    w1_d = nc.declare_dram_parameter("w1", [FIN, H], f32, isOutput=False)
    b1_d = nc.declare_dram_parameter("b1", [H, 1], f32, isOutput=False)
    wp1_d = nc.declare_dram_parameter("wp1", [H, K1], f32, isOutput=False)
    bp1_d = nc.declare_dram_parameter("bp1", [K1, 1], f32, isOutput=False)
    wrel1_d = nc.declare_dram_parameter("wrel1", [H, H], f32, isOutput=False)
    brel1_d = nc.declare_dram_parameter("brel1", [1, H], f32, isOutput=False)
    wroot1_d = nc.declare_dram_parameter("wroot1", [H, H], f32, isOutput=False)
    wp2a_d = nc.declare_dram_parameter("wp2aug", [H + 1, K2], f32, isOutput=False)
    ident_d = nc.declare_dram_parameter("ident", [128, 128], f32, isOutput=False)

    o_x2t = nc.declare_dram_parameter("o_x2t", [BL, H, N], f32, isOutput=True)
    o_s2 = nc.declare_dram_parameter("o_s2", [BL, N, K2], f32, isOutput=True)
    o_pt = nc.declare_dram_parameter("o_pt", [BL, K2, N], f32, isOutput=True)
    o_df2 = nc.declare_dram_parameter("o_df2", [BL, 1, N], f32, isOutput=True)
    o_ss1 = nc.declare_dram_parameter("o_ss1", [BL, K1, K1], f32, isOutput=True)
    o_aux = nc.declare_dram_parameter("o_aux", [BL, K1, 8], f32, isOutput=True)
    o_vb = nc.declare_dram_parameter("o_vb", [BL, 1, 8], f32, isOutput=True)

    with TileContext(nc) as tc:
        with (
            tc.tile_pool(name="consts", bufs=1) as consts,
            tc.tile_pool(name="abig", bufs=3) as abig,
            tc.tile_pool(name="atr", bufs=1) as p_atr,
            tc.tile_pool(name="anew", bufs=2) as p_anew,
            tc.tile_pool(name="acts", bufs=1) as acts,
            tc.tile_pool(name="stage", bufs=1) as stage,
            tc.tile_pool(name="scr", bufs=2) as scr,
            tc.tile_pool(name="ps_tr", bufs=3, space="PSUM") as ps_tr,
            tc.tile_pool(name="ps_d2", bufs=2, space="PSUM") as ps_d2,
            tc.tile_pool(name="ps_acc", bufs=2, space="PSUM") as ps_acc,
            tc.tile_pool(name="ps_mm", bufs=1, space="PSUM") as ps_mm,
        ):
            ident = consts.tile([128, 128], f32, tag="ident")
            nc.sync.dma_start(out=ident[:], in_=ident_d[:])

            def rounded_const(name, shape, src_dram):
                tf = consts.tile(shape, f32, tag=name + "f")
                nc.sync.dma_start(out=tf[:], in_=src_dram[:])
                tr = consts.tile(shape, f32r, tag=name)
                nc.vector.tensor_copy(tr[:], tf[:])
                return tr

            w1 = rounded_const("w1", [FIN, H], w1_d)
            wp1 = rounded_const("wp1", [H, K1], wp1_d)
            wrel1 = rounded_const("wrel1", [H, H], wrel1_d)
            wroot1 = rounded_const("wroot1", [H, H], wroot1_d)
            brel1 = rounded_const("brel1", [1, H], brel1_d)
            wp2a = rounded_const("wp2a", [H + 1, K2], wp2a_d)
            b1c = consts.tile([H, 1], f32, tag="b1c")
            nc.sync.dma_start(out=b1c[:], in_=b1_d[:])
            bp1c = consts.tile([K1, 1], f32, tag="bp1c")
            nc.sync.dma_start(out=bp1c[:], in_=bp1_d[:])

            ones_f = consts.tile([1, N], f32, tag="ones_f")
            nc.vector.memset(ones_f[:], 1.0)
            ones_row = consts.tile([1, N], f32r, tag="ones_row")
            nc.vector.tensor_copy(ones_row[:], ones_f[:])
            ones64f = consts.tile([K1, 1], f32, tag="ones64f")
            nc.vector.memset(ones64f[:], 1.0)
            ones64 = consts.tile([K1, 1], f32r, tag="ones64")
            nc.vector.tensor_copy(ones64[:], ones64f[:])
            onescol = consts.tile([128, 1], f32, tag="onescol")
            nc.vector.memset(onescol[:], 1.0)

            for g in range(BL):
                # ---- X0^T (feature-major input) ----
                x0t = acts.tile([FIN, N], f32r, tag="x0t")
                for j in range(NT):
                    x0 = abig.tile([128, FIN], f32, tag="x0")
                    nc.sync.dma_start(out=x0[:], in_=x_loc[g, j * 128 : (j + 1) * 128, :])
                    pt_ = ps_tr.tile([128, 128], f32, tag="ptr")
                    nc.tensor.transpose(pt_[:], x0[:], ident[:])
                    nc.vector.tensor_copy(x0t[:, j * 128 : (j + 1) * 128], pt_[:])

                # ---- X1^T = W1^T X0^T + b1 (staged f32, then rounded) ----
                x1t_f = stage.tile([H + 1, N], f32, tag="x1t_f")
                nc.vector.memset(x1t_f[H : H + 1, :], 1.0)
                for c in range(2):
                    pm = ps_mm.tile([H, 512], f32, tag="pmm")
                    nc.tensor.matmul(
                        pm[:], w1[:], x0t[:, c * 512 : (c + 1) * 512],
                        start=True, stop=True,
                    )
                    nc.vector.tensor_scalar_add(
                        x1t_f[0:H, c * 512 : (c + 1) * 512], pm[:], b1c[:]
                    )
                x1taug = acts.tile([H + 1, N], f32r, tag="x1taug")
                nc.vector.tensor_copy(x1taug[:], x1t_f[:])

                # ---- S^T = tanh(Wp1^T X1^T + bp1); colsum via accum ----
                st_f = stage.tile([K1 + 2, N], f32, tag="st_f")
                sts_f = stage.tile([K1 + 2, N], f32, tag="sts_f")
                aux = scr.tile([K1, 8], f32, tag="aux")
                for c in range(2):
                    pm = ps_mm.tile([K1, 512], f32, tag="pmm")
                    nc.tensor.matmul(
                        pm[:], wp1[:], x1taug[0:H, c * 512 : (c + 1) * 512],
                        start=True, stop=True,
                    )
                    nc.scalar.activation(
                        st_f[0:K1, c * 512 : (c + 1) * 512], pm[:], AF.Tanh,
                        bias=bp1c[:], accum_out=aux[:, c : c + 1],
                    )
                nc.vector.tensor_scalar_mul(sts_f[0:K1, :], st_f[0:K1, :], -2.0)

                # rn2 = sum_k S^2 as a [1,N] row (partition 0)
                stsq = stage.tile([K1, N], f32r, tag="stsq")
                nc.vector.tensor_mul(stsq[:], st_f[0:K1, :], st_f[0:K1, :])
                rn2_sb = stage.tile([1, N], f32, tag="rn2_sb")
                for c in range(2):
                    pm = ps_mm.tile([1, 512], f32, tag="pmm")
                    nc.tensor.matmul(
                        pm[:], ones64[:], stsq[:, c * 512 : (c + 1) * 512],
                        start=True, stop=True,
                    )
                    nc.scalar.activation(
                        rn2_sb[:, c * 512 : (c + 1) * 512], pm[:], AF.Copy
                    )
                # special rows: st = [S; 1; rn2], sts = [-2S; rn2; 1]
                nc.vector.memset(st_f[K1 : K1 + 1, :], 1.0)
                nc.sync.dma_start(out=st_f[K1 + 1 : K1 + 2, :], in_=rn2_sb[:])
                nc.sync.dma_start(out=sts_f[K1 : K1 + 1, :], in_=rn2_sb[:])
                nc.sync.dma_start(out=sts_f[K1 + 1 : K1 + 2, :], in_=ones_f[:])
                st = acts.tile([K1 + 2, N], f32r, tag="st")
                sts = acts.tile([K1 + 2, N], f32r, tag="sts")
                nc.vector.tensor_copy(st[:], st_f[:])
                nc.vector.tensor_copy(sts[:], sts_f[:])

                # ---- S node-major blocks + ones col (Zstat) ----
                zstat = []
                for j in range(NT):
                    zs = acts.tile([128, K1 + 1], f32r, tag=f"zstat{j}")
                    pt_ = ps_tr.tile([128, 128], f32, tag="ptr")
                    nc.tensor.transpose(
                        pt_[0:128, 0:K1],
                        st_f[0:K1, j * 128 : (j + 1) * 128],
                        ident[0:K1, 0:K1],
                    )
                    nc.vector.tensor_copy(zs[:, 0:K1], pt_[0:128, 0:K1])
                    nc.vector.tensor_copy(zs[:, K1 : K1 + 1], onescol[:])
                    zstat.append(zs)

                # ---- A^T tiles (PE transpose) + Asq ----
                atr = []
                for j in range(NT):
                    at_j = p_atr.tile([128, N], f32r, tag=f"atr{j}")
                    atr.append(at_j)
                for i in range(NT):
                    a_nat = abig.tile([128, N], f32, tag="a_nat")
                    nc.sync.dma_start(
                        out=a_nat[:], in_=a_loc[g, i * 128 : (i + 1) * 128, :]
                    )
                    for j in range(NT):
                        pt_ = ps_tr.tile([128, 128], f32, tag="ptr")
                        nc.tensor.transpose(
                            pt_[:], a_nat[:, j * 128 : (j + 1) * 128], ident[:]
                        )
                        nc.scalar.activation(
                            atr[j][:, i * 128 : (i + 1) * 128], pt_[:], AF.Copy
                        )

                # ---- ASt = [S_nm|1]^T A^T : rows 0..63 = (A S)^T, row 64 = dflat
                dfl = stage.tile([K1 + 1, N], f32, tag="dfl")
                vb = scr.tile([K1 + 1, 8], f32, tag="vb")
                for c in range(2):
                    pa = ps_acc.tile([K1 + 1, 512], f32, tag="pacc")
                    for j in range(NT):
                        nc.tensor.matmul(
                            pa[:], zstat[j][:], atr[j][:, c * 512 : (c + 1) * 512],
                            start=(j == 0), stop=(j == NT - 1),
                        )
                    ttr_dump = scr.tile([K1, 512], f32, tag="ttr_dump")
                    nc.vector.tensor_mul(
                        ttr_dump[:], pa[0:K1, :],
                        st[0:K1, c * 512 : (c + 1) * 512].bitcast(f32),
                    )
                    nc.vector.reduce_sum(
                        aux[:, 2 + c : 3 + c], ttr_dump[:], axis=mybir.AxisListType.X
                    )
                    nc.scalar.activation(
                        dfl[K1 : K1 + 1, c * 512 : (c + 1) * 512],
                        pa[K1 : K1 + 1, :],
                        AF.Copy, accum_out=vb[K1 : K1 + 1, c : c + 1],
                    )
                # basedot = sum(dflat * rn2) ; rn2 lives at sts row 64
                bd_dump = stage.tile([K1 + 1, N], f32, tag="bd_dump")
                nc.vector.tensor_mul(
                    bd_dump[K1 : K1 + 1, :], dfl[K1 : K1 + 1, :],
                    sts[K1 : K1 + 1, :].bitcast(f32),
                )
                nc.vector.reduce_sum(
                    vb[K1 : K1 + 1, 2:3], bd_dump[K1 : K1 + 1, :],
                    axis=mybir.AxisListType.X,
                )
                # vol = volp0 + volp1 + N*EPS -> invvol2 broadcast [128,1]
                nc.vector.tensor_tensor(
                    out=vb[K1 : K1 + 1, 3:4], in0=vb[K1 : K1 + 1, 0:1],
                    in1=vb[K1 : K1 + 1, 1:2], op=ALU.add,
                )
                nc.vector.tensor_scalar_add(
                    vb[K1 : K1 + 1, 4:5], vb[K1 : K1 + 1, 3:4], float(N) * EPS
                )
                nc.sync.dma_start(out=o_vb[g], in_=vb[K1 : K1 + 1, :])
                volp0 = scr.tile([1, 2], f32, tag="volp0")
                nc.sync.dma_start(out=volp0[:, 0:1], in_=vb[K1 : K1 + 1, 4:5])
                pv = ps_mm.tile([128, 1], f32, tag="pmm")
                nc.tensor.matmul(
                    pv[:], ones_f[0:1, 0:128], volp0[:, 0:1], start=True, stop=True
                )
                rcp = scr.tile([128, 2], f32, tag="rcp")
                nc.vector.reciprocal(rcp[:, 0:1], pv[:])
                nc.vector.tensor_mul(rcp[:, 1:2], rcp[:, 0:1], rcp[:, 0:1])

                # ---- d2 -> clamp*Asq -> sqrt -> Anew^T (f32r) ----
                anew = []
                for j in range(NT):
                    an_j = p_anew.tile([128, N], f32r, tag=f"anew{j}")
                    anew.append(an_j)
                for j in range(NT):
                    for c in range(2):
                        pd = ps_d2.tile([128, 512], f32, tag="pd2")
                        nc.tensor.matmul(
                            pd[:],
                            sts[:, j * 128 : (j + 1) * 128],
                            st[:, c * 512 : (c + 1) * 512],
                            start=True, stop=True,
                        )
                        tq = scr.tile([128, 512], f32, tag="tq")
                        nc.vector.scalar_tensor_tensor(
                            out=tq[:], in0=pd[:], scalar=0.0,
                            in1=atr[j][:, c * 512 : (c + 1) * 512].bitcast(f32),
                            op0=ALU.max, op1=ALU.mult,
                        )
                        tq2 = scr.tile([128, 512], f32, tag="tq2")
                        nc.vector.tensor_mul(
                            tq2[:], tq[:],
                            atr[j][:, c * 512 : (c + 1) * 512].bitcast(f32),
                        )
                        nc.scalar.activation(
                            anew[j][:, c * 512 : (c + 1) * 512], tq2[:], AF.Sqrt,
                            scale=rcp[:, 1:2],
                        )

                # ---- X1 node-major aug blocks ----
                x1nm = []
                for j in range(NT):
                    xn = acts.tile([128, H + 1], f32r, tag=f"x1nm{j}")
                    pt_ = ps_tr.tile([128, 128], f32, tag="ptr")
                    nc.tensor.transpose(
                        pt_[0:128, 0 : H + 1],
                        x1t_f[:, j * 128 : (j + 1) * 128],
                        ident[0 : H + 1, 0 : H + 1],
                    )
                    nc.vector.tensor_copy(xn[:], pt_[0:128, 0 : H + 1])
                    x1nm.append(xn)

                # ---- R^T = [X1|1]_nm^T Anew^T ----
                rt = acts.tile([H + 1, N], f32r, tag="rt")
                for c in range(2):
                    pa = ps_acc.tile([H + 1, 512], f32, tag="pacc")
                    for j in range(NT):
                        nc.tensor.matmul(
                            pa[:], x1nm[j][:], anew[j][:, c * 512 : (c + 1) * 512],
                            start=(j == 0), stop=(j == NT - 1),
                        )
                    nc.vector.tensor_copy(rt[:, c * 512 : (c + 1) * 512], pa[:])
                nc.sync.dma_start(out=o_df2[g], in_=rt[H : H + 1, :].bitcast(f32))

                # ---- X2^T = Wrel1^T R^T + brel1 + Wroot1^T X1^T ----
                x2t_f = stage.tile([H + 1, N], f32, tag="x2t_f")
                nc.vector.memset(x2t_f[H : H + 1, :], 1.0)
                for c in range(2):
                    pm = ps_mm.tile([H, 512], f32, tag="pmm")
                    sl = slice(c * 512, (c + 1) * 512)
                    nc.tensor.matmul(pm[:], wrel1[:], rt[0:H, sl], start=True, stop=False)
                    nc.tensor.matmul(
                        pm[:], brel1[:], ones_row[:, sl], start=False, stop=False
                    )
                    nc.tensor.matmul(
                        pm[:], wroot1[:], x1taug[0:H, sl], start=False, stop=True
                    )
                    nc.vector.tensor_copy(x2t_f[0:H, sl], pm[:])
                x2taug = acts.tile([H + 1, N], f32r, tag="x2taug")
                nc.vector.tensor_copy(x2taug[:], x2t_f[:])
                nc.sync.dma_start(out=o_x2t[g], in_=x2t_f[0:H, :])

                # ---- S2 = softmax_nodes(X2 Wp2 + bp2), node-major ----
                s2all = acts.tile([128, NT * K2], f32r, tag="s2all")
                for j in range(NT):
                    pm = ps_mm.tile([128, K2], f32, tag="pmm")
                    nc.tensor.matmul(
                        pm[:], x2taug[:, j * 128 : (j + 1) * 128], wp2a[:],
                        start=True, stop=True,
                    )
                    en = scr.tile([128, K2], f32, tag="en")
                    se = scr.tile([128, 2], f32, tag="se")
                    nc.scalar.activation(en[:], pm[:], AF.Exp, accum_out=se[:, 0:1])
                    nc.vector.reciprocal(se[:, 1:2], se[:, 0:1])
                    nc.vector.tensor_scalar_mul(
                        s2all[:, j * K2 : (j + 1) * K2], en[:], se[:, 1:2]
                    )
                    nc.sync.dma_start(
                        out=o_s2[g, j * 128 : (j + 1) * 128, :],
                        in_=s2all[:, j * K2 : (j + 1) * K2].bitcast(f32),
                    )

                # ---- P^T = S2_nm^T Anew^T ----
                ptile = acts.tile([K2, N], f32, tag="ptile")
                for c in range(2):
                    pa = ps_acc.tile([K2, 512], f32, tag="pacc")
                    for j in range(NT):
                        nc.tensor.matmul(
                            pa[:], s2all[:, j * K2 : (j + 1) * K2],
                            anew[j][:, c * 512 : (c + 1) * 512],
                            start=(j == 0), stop=(j == NT - 1),
                        )
                    nc.vector.tensor_copy(ptile[:, c * 512 : (c + 1) * 512], pa[:])
                nc.sync.dma_start(out=o_pt[g], in_=ptile[:])

                # ---- SS1 = S^T S ----
                pss = ps_mm.tile([K1, K1], f32, tag="pmm")
                for j in range(NT):
                    nc.tensor.matmul(
                        pss[:], zstat[j][:, 0:K1], zstat[j][:, 0:K1],
                        start=(j == 0), stop=(j == NT - 1),
                    )
                ss1 = scr.tile([K1, K1], f32, tag="ss1")
                nc.vector.tensor_copy(ss1[:], pss[:])
                nc.sync.dma_start(out=o_ss1[g], in_=ss1[:])
                nc.sync.dma_start(out=o_aux[g], in_=aux[:])

    _split_excess_waits(nc)
    return nc


def kernel(**inputs):
    x = np.asarray(inputs["x"], dtype=np.float32)
    adj = np.asarray(inputs["adj"], dtype=np.float32)
    mask = np.asarray(inputs["mask"])
    m = mask.astype(np.float32)  # all-ones in this problem; applied to x anyway

    if "nc" not in _CACHE:
        _CACHE["nc"] = _build()
    nc = _CACHE["nc"]

    wk = {
        "w1": np.ascontiguousarray(inputs["W_lin1"], dtype=np.float32),
        "b1": np.ascontiguousarray(np.asarray(inputs["b_lin1"], np.float32)[:, None]),
        "wp1": np.ascontiguousarray(inputs["W_pool1"], dtype=np.float32),
        "bp1": np.ascontiguousarray(np.asarray(inputs["b_pool1"], np.float32)[:, None]),
        "wrel1": np.ascontiguousarray(inputs["Wrel1"], dtype=np.float32),
        "brel1": np.ascontiguousarray(np.asarray(inputs["brel1"], np.float32)[None, :]),
        "wroot1": np.ascontiguousarray(inputs["Wroot1"], dtype=np.float32),
        "wp2aug": np.ascontiguousarray(
            np.vstack(
                [
                    np.asarray(inputs["W_pool2"], np.float32),
                    np.asarray(inputs["b_pool2"], np.float32)[None, :],
                ]
            )
        ),
        "ident": np.eye(128, dtype=np.float32),
    }
    xm = x * m[:, :, None]
    in_maps = []
    for c in range(NC):
        sl = slice(c * BL, (c + 1) * BL)
        in_maps.append(
            dict(
                a_loc=np.ascontiguousarray(adj[sl]),
                x_loc=np.ascontiguousarray(xm[sl]),
                **wk,
            )
        )

    results = run_bass_kernel_spmd(nc, in_maps, list(range(NC))).results

    # ---------------- host tail ----------------
    f64 = np.float64
    Wrel2 = np.asarray(inputs["Wrel2"], f64)
    brel2 = np.asarray(inputs["brel2"], f64)
    Wroot2 = np.asarray(inputs["Wroot2"], f64)
    W_lin2 = np.asarray(inputs["W_lin2"], f64)
    b_lin2 = np.asarray(inputs["b_lin2"], f64)
    W_lin3 = np.asarray(inputs["W_lin3"], f64)
    b_lin3 = np.asarray(inputs["b_lin3"], f64)

    logits = np.zeros((B, OUT), np.float32)
    ct_ratios, o1sq, mincuts, o2s = [], [], [], []
    i64 = np.eye(K1, dtype=f64)
    i16 = np.eye(K2, dtype=f64)
    for c in range(NC):
        r = results[c]
        for g in range(BL):
            bidx = c * BL + g
            auxv = r["o_aux"][g].astype(f64)
            vbv = r["o_vb"][g][0].astype(f64)
            colsum = auxv[:, 0] + auxv[:, 1]
            trAS = auxv[:, 2].sum() + auxv[:, 3].sum()
            basedot = vbv[2]
            epsterm = EPS * np.sum(colsum**2)
            ct_num = basedot + epsterm - trAS
            ct_den = basedot + epsterm + EPS
            ct_ratios.append(ct_num / ct_den)
            ss1 = r["o_ss1"][g].astype(f64)
            diff1 = ss1 / np.sqrt(np.sum(ss1 * ss1)) - i64
            o1sq.append(np.sum(diff1 * diff1))

            X2 = r["o_x2t"][g].astype(f64).T  # [N, H]
            S2 = r["o_s2"][g].astype(f64)  # [N, K2]
            P = r["o_pt"][g].astype(f64).T  # [N, K2]
            dflat2 = r["o_df2"][g][0].astype(f64) + EPS
            OUT2 = S2.T @ X2
            OA = S2.T @ P
            mincut_num = np.trace(OA)
            mincut_den = np.sum(dflat2 * np.sum(S2 * S2, axis=1))
            mincuts.append(-(mincut_num / mincut_den))
            SS2 = S2.T @ S2
            diff2 = SS2 / np.sqrt(np.sum(SS2 * SS2)) - i16 / np.sqrt(float(K2))
            o2s.append(np.sqrt(np.sum(diff2 * diff2)))

            OAp = OA * (1.0 - i16)
            dk = np.sqrt(OAp.sum(axis=-1) + EPS) + EPS
            OAn = OAp / dk[None, :] / dk[:, None]
            X3 = (OAn @ OUT2) @ Wrel2 + brel2 + OUT2 @ Wroot2
            gsum = X3.sum(axis=0)
            h = np.maximum(gsum @ W_lin2 + b_lin2, 0.0)
            lg = h @ W_lin3 + b_lin3
            lg = lg - (np.log(np.sum(np.exp(lg - lg.max()))) + lg.max())
            logits[bidx] = lg.astype(np.float32)

    loss1 = np.float32(np.mean(ct_ratios) + np.sqrt(np.sum(o1sq)))
    loss2 = np.float32(np.mean(mincuts) + np.mean(o2s))
    return logits, loss1, loss2
